# revision 11
# baseline (speedup 1.0000x reference)
"""Causal attention kernel for 8 Trainium2 NeuronCores.

Problem: x[4, 4096, 512] @ {Wq,Wk,Wv}[512, 128] -> causal attention -> [4, 4096, 128].

Sharding: 2 cores per batch, interleaved over KEY chunks. Core c = 2b+p
(batch b, parity p) owns key chunks {2i+p : i=0..15} (chunks of 128 keys),
and computes, for every query block of its batch, the partial softmax
numerator and denominator over its keys. The host sums the two partials and
divides. Causality makes query block qg (512 rows) attend key chunks
0..4qg+3, of which each parity owns exactly 2qg+2 -> both cores run the
identical program (exact load balance); only the last two local chunks of
each block are masked (multiplicative {0,1} mask, input-supplied).

On-device layout: scores are computed transposed, S^T[key, q]:
  - projections run in fp8e4 DoubleRow (2 contraction tiles per pass),
    outputs cast to bf16 (K^T, Q^T) / fp8 (V)
  - S^T chunk = bf16 matmul(lhsT=K^T[:, chunk], rhs=Q^T[:, qblock])
  - chunks are processed in PAIRS: both score matmuls land in one 2-bank
    PSUM tile; ONE ScalarE exp (scale=1/sqrt(d)) covers the pair,
    amortizing the fixed activation overhead; output E is fp8
  - the causal mask is additive (-1e9), pre-biased into the pair's PSUM
    by DVE BEFORE the score matmuls accumulate (start=False), so masking
    stays off the exp->AV critical chain (only the last pair of a block)
  - numerator: ONE fp8 DoubleRow matmul per pair (lhsT = V chunk pair)
  - denominator: ONE fp8 DoubleRow matmul per pair with a one-hot
    stationary [128, 2, 8] that routes the sum into row qg of a single
    persistent PSUM bank [8, 512] holding all 8 blocks' denominators
  - query block 0 (rows with few attended keys) runs E/V in bf16 to keep
    fp8 quantization out of the near-copy early rows; everything else
    tolerates fp8 (softmax-weight averaging suppresses the error)
"""

import math

import numpy as np

B, S, DIN, DOUT = 4, 4096, 512, 128
NCORES = 8
TQ = 512            # query block size
NQB = S // TQ       # 8 query blocks per batch
KC = 128            # key chunk size
NKLOC = S // KC // 2  # 16 key chunks owned per core
SK = NKLOC * KC     # 2048 owned keys
NDC = DIN // 128    # 4 contraction chunks
RSQRT_D = 1.0 / math.sqrt(float(DOUT))

_cache = {}


def _build_nc():
    import concourse.bacc as bacc
    import concourse.mybir as mybir
    import concourse.tile as tile

    f32 = mybir.dt.float32
    bf = mybir.dt.bfloat16
    f8 = mybir.dt.float8e4
    DR = mybir.MatmulPerfMode.DoubleRow
    EXP = mybir.ActivationFunctionType.Exp

    nc = bacc.Bacc(None, target_bir_lowering=False, debug=False)

    # ---- DRAM parameters ----
    xq8_d = nc.declare_dram_parameter("xq8", [128, NDC, S], f8, isOutput=False)
    xk8_d = nc.declare_dram_parameter("xk8", [128, NDC, SK], f8, isOutput=False)
    xv16_d = nc.declare_dram_parameter("xv16", [128, NDC, 2 * KC], bf, isOutput=False)
    w8_d = nc.declare_dram_parameter("w8", [128, 3, NDC, DOUT], f8, isOutput=False)
    wv16_d = nc.declare_dram_parameter("wv16", [128, NDC, DOUT], bf, isOutput=False)
    mkA_d = nc.declare_dram_parameter("mkA", [128, 2, TQ], bf, isOutput=False)
    oh8_d = nc.declare_dram_parameter("oh8", [128, 2, 8 * NQB], f8, isOutput=False)
    oh16_d = nc.declare_dram_parameter("oh16", [128, 8], bf, isOutput=False)
    numT = nc.declare_dram_parameter("numT", [DOUT, S], f32, isOutput=True)
    den = nc.declare_dram_parameter("den", [NQB, TQ], f32, isOutput=True)

    with tile.TileContext(nc) as tc:
        with (
            tc.tile_pool(name="persist", bufs=1) as persist,
            tc.tile_pool(name="pp", bufs=1, space="PSUM") as pp,
            tc.tile_pool(name="ps2", bufs=2, space="PSUM") as ps2,
            tc.tile_pool(name="pso", bufs=2, space="PSUM") as pso,
            tc.tile_pool(name="psd", bufs=1, space="PSUM") as psd,
            tc.tile_pool(name="et", bufs=4) as et,
            tc.tile_pool(name="ot", bufs=2) as ot,
            tc.tile_pool(name="dt", bufs=1) as dt_pool,
        ):
            # ---- resident SBUF tensors ----
            xq8_t = persist.tile([128, NDC, S], f8, tag="xq8")
            xk8_t = persist.tile([128, NDC, SK], f8, tag="xk8")
            xv16_t = persist.tile([128, NDC, 2 * KC], bf, tag="xv16")
            w8_t = persist.tile([128, 3, NDC, DOUT], f8, tag="w8")
            wv16_t = persist.tile([128, NDC, DOUT], bf, tag="wv16")
            mkA_t = persist.tile([128, 2, TQ], bf, tag="mkA")
            oh8_t = persist.tile([128, 2, 8 * NQB], f8, tag="oh8")
            oh16_t = persist.tile([128, 8], bf, tag="oh16")
            qT = persist.tile([128, S], bf, tag="qT")
            kT = persist.tile([128, SK], bf, tag="kT")
            v8_t = persist.tile([128, NKLOC, DOUT], f8, tag="v8")
            v16_t = persist.tile([128, 2, DOUT], bf, tag="v16")

            # ---- input DMA: spread across three issue queues so transfers
            # overlap. K-path (gates everything) on the sync (SP) ring,
            # Q-path on the scalar ring (idle until the first exp), small
            # tensors on gpsimd.
            nc.sync.dma_start(out=w8_t[:], in_=w8_d[:])
            nc.sync.dma_start(out=xk8_t[:, :, 0:512], in_=xk8_d[:, :, 0:512])
            nc.scalar.dma_start(out=xq8_t[:, :, 0:512], in_=xq8_d[:, :, 0:512])
            nc.sync.dma_start(out=xk8_t[:, :, 512:1024], in_=xk8_d[:, :, 512:1024])
            nc.gpsimd.dma_start(out=xv16_t[:], in_=xv16_d[:])
            nc.gpsimd.dma_start(out=wv16_t[:], in_=wv16_d[:])
            nc.gpsimd.dma_start(out=mkA_t[:], in_=mkA_d[:])
            nc.gpsimd.dma_start(out=oh8_t[:], in_=oh8_d[:])
            nc.gpsimd.dma_start(out=oh16_t[:], in_=oh16_d[:])
            nc.scalar.dma_start(out=xq8_t[:, :, 512:1024], in_=xq8_d[:, :, 512:1024])
            nc.sync.dma_start(out=xk8_t[:, :, 1024:SK], in_=xk8_d[:, :, 1024:SK])
            nc.scalar.dma_start(out=xq8_t[:, :, 1024:2048], in_=xq8_d[:, :, 1024:2048])
            nc.scalar.dma_start(out=xq8_t[:, :, 2048:S], in_=xq8_d[:, :, 2048:S])

            def kproj(g):  # K^T for owned keys [512g, 512g+512)
                ps = pp.tile([128, 512], f32, tag="pp", name=f"ppk{g}")
                for j in (0, 1):
                    nc.tensor.matmul(
                        ps[:],
                        w8_t[:, 1, 2 * j:2 * j + 2, :],
                        xk8_t[:, 2 * j:2 * j + 2, 512 * g:512 * (g + 1)],
                        start=(j == 0),
                        stop=(j == 1),
                        perf_mode=DR,
                    )
                nc.vector.tensor_copy(kT[:, 512 * g:512 * (g + 1)], ps[:])

            def qproj(g):  # Q^T for queries [512g, 512g+512)
                ps = pp.tile([128, 512], f32, tag="pp", name=f"ppq{g}")
                for j in (0, 1):
                    nc.tensor.matmul(
                        ps[:],
                        w8_t[:, 0, 2 * j:2 * j + 2, :],
                        xq8_t[:, 2 * j:2 * j + 2, 512 * g:512 * (g + 1)],
                        start=(j == 0),
                        stop=(j == 1),
                        perf_mode=DR,
                    )
                nc.vector.tensor_copy(qT[:, 512 * g:512 * (g + 1)], ps[:])

            def vproj(g):  # V for local chunks 4g..4g+3, fp8
                ps = pp.tile([128, 4, DOUT], f32, tag="pp", name=f"ppv{g}")
                for c in range(4):
                    ck = 4 * g + c
                    for j in (0, 1):
                        nc.tensor.matmul(
                            ps[:, c, :],
                            xk8_t[:, 2 * j:2 * j + 2, KC * ck:KC * (ck + 1)],
                            w8_t[:, 2, 2 * j:2 * j + 2, :],
                            start=(j == 0),
                            stop=(j == 1),
                            perf_mode=DR,
                        )
                nc.vector.tensor_copy(v8_t[:, 4 * g:4 * (g + 1), :], ps[:])

            def v16proj():  # bf16 V for local chunks 0,1 (block-0 accuracy)
                ps = pp.tile([128, 2, DOUT], f32, tag="pp", name="ppv16")
                for c in (0, 1):
                    for t in range(NDC):
                        nc.tensor.matmul(
                            ps[:, c, :],
                            xv16_t[:, t, KC * c:KC * (c + 1)],
                            wv16_t[:, t, :],
                            start=(t == 0),
                            stop=(t == NDC - 1),
                        )
                nc.vector.tensor_copy(v16_t[:], ps[:])

            pd = psd.tile([8, TQ], f32, tag="pd", name="pd")

            def attn_block(qg, first_den=False, last_den_blk=False):
                npairs = qg + 1
                po = pso.tile([128, TQ], f32, tag="po", name=f"po{qg}")
                for i in range(npairs):
                    masked = i == npairs - 1
                    pair = ps2.tile([128, 2, TQ], f32, tag="ps2", name=f"ps{qg}_{i}")
                    if masked:
                        # additive causal mask pre-biased into PSUM, off the
                        # exp->AV critical chain
                        nc.vector.tensor_copy(pair[:], mkA_t[:])
                    for c in (0, 1):
                        ck = 2 * i + c
                        nc.tensor.matmul(
                            pair[:, c, :],
                            kT[:, KC * ck:KC * (ck + 1)],
                            qT[:, TQ * qg:TQ * (qg + 1)],
                            start=not masked,
                            stop=True,
                        )
                    edt = bf if qg == 0 else f8
                    etag = "e16" if qg == 0 else "e8"
                    e = et.tile([128, 2, TQ], edt, tag=etag, name=f"e{qg}_{i}")
                    nc.scalar.activation(e[:], pair[:], EXP, scale=RSQRT_D)
                    last_den = last_den_blk and i == npairs - 1
                    if qg == 0:
                        for c in (0, 1):
                            nc.tensor.matmul(
                                po[:],
                                v16_t[:, c, :],
                                e[:, c, :],
                                start=(c == 0),
                                stop=(c == 1),
                            )
                            nc.tensor.matmul(
                                pd[:],
                                oh16_t[:],
                                e[:, c, :],
                                start=(c == 0),
                                stop=False,
                                skip_group_check=True,
                            )
                    else:
                        nc.tensor.matmul(
                            po[:],
                            v8_t[:, 2 * i:2 * i + 2, :],
                            e[:],
                            start=(i == 0),
                            stop=(i == npairs - 1),
                            perf_mode=DR,
                        )
                        nc.tensor.matmul(
                            pd[:],
                            oh8_t[:, :, 8 * qg:8 * (qg + 1)],
                            e[:],
                            start=False,
                            stop=last_den,
                            perf_mode=DR,
                            skip_group_check=True,
                        )
                o = ot.tile([128, TQ], f32, tag="o", name=f"o{qg}")
                nc.vector.tensor_copy(o[:], po[:])
                nc.gpsimd.dma_start(out=numT[:, TQ * qg:TQ * (qg + 1)], in_=o[:])

            # ---- schedule: projections interleaved as tensor-engine filler;
            # big blocks early (while filler exists), small block last so the
            # end-of-kernel scalar-bound drain is short
            kproj(0)
            v16proj()
            vproj(0)
            qproj(0)
            attn_block(0, first_den=True)
            kproj(1)
            vproj(1)
            qproj(1)
            attn_block(1)
            kproj(2)
            vproj(2)
            qproj(2)
            attn_block(2)
            kproj(3)
            vproj(3)
            qproj(3)
            attn_block(3)
            qproj(7)
            attn_block(7)
            qproj(6)
            attn_block(6)
            qproj(5)
            attn_block(5)
            qproj(4)
            attn_block(4, last_den_blk=True)

            d = dt_pool.tile([8, TQ], f32, tag="d", name="d")
            nc.vector.tensor_copy(d[:], pd[:])
            nc.gpsimd.dma_start(out=den[:, :], in_=d[:])

    nc.finalize()
    return nc


def _owned_keys(par):
    return np.concatenate(
        [np.arange((2 * i + par) * KC, (2 * i + par) * KC + KC) for i in range(NKLOC)]
    )


def _build_masksA(par):
    # additive causal masks for the last pair of each query block:
    # pair-half j in {0,1} is global chunk 4qg+2j+par; element [k, q]
    # allowed iff 128*(2j+par) + k <= q (same for every block)
    j = np.arange(2)[:, None, None]
    k = np.arange(KC)[None, :, None]
    q = np.arange(TQ)[None, None, :]
    allowed = (KC * (2 * j + par) + k) <= q
    return np.where(allowed, np.float32(0.0), np.float32(-1.0e9))  # [2, 128, 512]


def _get_nc():
    if "nc" not in _cache:
        _cache["nc"] = _build_nc()
    return _cache["nc"]


def _pack_pm(a):
    # [DIN, cols] -> partition-major [128, DIN//128, cols]
    return np.ascontiguousarray(a.reshape(DIN // 128, 128, a.shape[1]).transpose(1, 0, 2))


def _prepare_in_maps(x, Wq, Wk, Wv):
    import ml_dtypes

    f8 = ml_dtypes.float8_e4m3
    bf = ml_dtypes.bfloat16

    # [128, 3, NDC, DOUT]: w8[p, i, c, e] = W_i[128c + p, e]
    w8 = np.stack([_pack_pm(w).reshape(128, NDC, DOUT) for w in (Wq, Wk, Wv)], axis=1)
    w8 = np.ascontiguousarray(w8).astype(f8)
    wv16 = _pack_pm(Wv).astype(bf)

    # one-hot denominator routers
    oh8 = np.zeros((128, 2, 8 * NQB), dtype=np.float32)
    for qg in range(NQB):
        oh8[:, :, 8 * qg + qg] = 1.0
    oh8 = oh8.astype(f8)
    oh16 = np.zeros((128, 8), dtype=np.float32)
    oh16[:, 0] = 1.0
    oh16 = oh16.astype(bf)

    in_maps = []
    for c in range(NCORES):
        b, par = c // 2, c % 2
        xbt = x[b].T.astype(np.float32)
        ok = _owned_keys(par)
        m = _build_masksA(par)  # [2, 128, 512]
        mkA = np.ascontiguousarray(m.transpose(1, 0, 2)).astype(bf)
        in_maps.append({
            "xq8": _pack_pm(xbt).astype(f8),
            "xk8": _pack_pm(np.ascontiguousarray(xbt[:, ok])).astype(f8),
            "xv16": _pack_pm(np.ascontiguousarray(xbt[:, ok[:2 * KC]])).astype(bf),
            "w8": w8,
            "wv16": wv16,
            "mkA": mkA,
            "oh8": oh8,
            "oh16": oh16,
        })
    return in_maps


def _gather(results):
    out = np.empty((B, S, DOUT), dtype=np.float32)
    for b in range(B):
        r0, r1 = results[2 * b], results[2 * b + 1]
        num = r0["numT"].astype(np.float64).T + r1["numT"].astype(np.float64).T
        d = r0["den"].astype(np.float64).reshape(-1) + r1["den"].astype(
            np.float64
        ).reshape(-1)
        out[b] = (num / d[:, None]).astype(np.float32)
    return out


def kernel(**inputs):
    from concourse.bass_utils import run_bass_kernel_spmd

    x = np.asarray(inputs["x"], dtype=np.float32)
    Wq = np.asarray(inputs["Wq"], dtype=np.float32)
    Wk = np.asarray(inputs["Wk"], dtype=np.float32)
    Wv = np.asarray(inputs["Wv"], dtype=np.float32)

    nc = _get_nc()
    in_maps = _prepare_in_maps(x, Wq, Wk, Wv)
    res = run_bass_kernel_spmd(nc, in_maps, list(range(NCORES)))
    return _gather(res.results)


# revision 13
# speedup vs baseline: 1.0269x; 1.0269x over previous
"""Causal attention kernel for 8 Trainium2 NeuronCores.

Problem: x[4, 4096, 512] @ {Wq,Wk,Wv}[512, 128] -> causal attention -> [4, 4096, 128].

Sharding: 2 cores per batch, interleaved over KEY chunks. Core c = 2b+p
(batch b, parity p) owns keys {2j+p} and computes, for every query block of
its batch, the partial softmax numerator/denominator over its keys; the host
sums the two partials and divides. Causality gives both parities identical
per-block work (exact load balance, identical SPMD program).

Within every 512-query block, the sequence axis is HOST-PERMUTED per core to
"owned keys first" (pi(r) = 2r+par for r<256, 2(r-256)+(1-par) otherwise).
Queries and keys share the axis, so one fp8 x tensor serves all three
projections: K/V projections read the leading 256 columns of each block
(the owned keys) directly via strided APs -- no separate gathered copy. The
host un-permutes the output columns.

On-device layout: scores are computed transposed, S^T[key, q]:
  - projections run in fp8e4 DoubleRow (2 contraction k-tiles per pass),
    outputs cast to bf16 (K^T, Q^T) / fp8 (V)
  - S^T chunk = bf16 matmul(lhsT=K^T[:, chunk], rhs=Q^T[:, qblock])
  - chunks are processed in PAIRS: both score matmuls land in one 2-bank
    PSUM tile; ONE ScalarE exp (scale=1/sqrt(d)) covers the pair,
    amortizing the fixed activation overhead; output E is fp8
  - the causal mask (0/-240, fp8) is written into the pair's PSUM by an
    identity matmul BEFORE the scores accumulate (start=False), keeping
    masking entirely on the tensor queue (no cross-engine stall); after
    exp the masked scores flush to exactly 0 in fp8
  - numerator: ONE fp8 DoubleRow matmul per pair (lhsT = V chunk pair)
  - denominator: ONE fp8 DoubleRow matmul per pair with a one-hot
    stationary [128, 2, 8] routing the sum into row qg of a single
    persistent PSUM bank [8, 512] holding all 8 blocks' denominators
  - query block 0 runs E/V in bf16 (early rows are near-copies of V and
    cannot tolerate fp8 quantization; everywhere else softmax averaging
    suppresses it)
  - block order 0,1,2,3,7,6,5,4: big blocks run while projection filler
    exists; the kernel drains on a small block
"""

import math

import numpy as np

B, S, DIN, DOUT = 4, 4096, 512, 128
NCORES = 8
TQ = 512            # query block size
NQB = S // TQ       # 8 query blocks per batch
KC = 128            # key chunk size
NKLOC = S // KC // 2  # 16 key chunks owned per core
SK = NKLOC * KC     # 2048 owned keys
NDC = DIN // 128    # 4 contraction chunks
RSQRT_D = 1.0 / math.sqrt(float(DOUT))

_cache = {}


def _build_nc():
    import concourse.bacc as bacc
    import concourse.mybir as mybir
    import concourse.tile as tile

    f32 = mybir.dt.float32
    bf = mybir.dt.bfloat16
    f8 = mybir.dt.float8e4
    DR = mybir.MatmulPerfMode.DoubleRow
    EXP = mybir.ActivationFunctionType.Exp

    nc = bacc.Bacc(None, target_bir_lowering=False, debug=False)

    # ---- DRAM parameters ----
    xq8_d = nc.declare_dram_parameter("xq8", [128, NDC, S], f8, isOutput=False)
    xv16_d = nc.declare_dram_parameter("xv16", [128, NDC, 2 * KC], bf, isOutput=False)
    w8_d = nc.declare_dram_parameter("w8", [128, 3, NDC, DOUT], f8, isOutput=False)
    i8_d = nc.declare_dram_parameter("i8", [128, 128], f8, isOutput=False)
    wv16_d = nc.declare_dram_parameter("wv16", [128, NDC, DOUT], bf, isOutput=False)
    mk8_d = nc.declare_dram_parameter("mk8", [128, 2, TQ], f8, isOutput=False)
    oh8_d = nc.declare_dram_parameter("oh8", [128, 2, 8 * NQB], f8, isOutput=False)
    oh16_d = nc.declare_dram_parameter("oh16", [128, 8], bf, isOutput=False)
    numT = nc.declare_dram_parameter("numT", [DOUT, S], f32, isOutput=True)
    den = nc.declare_dram_parameter("den", [NQB, TQ], f32, isOutput=True)

    with tile.TileContext(nc) as tc:
        with (
            tc.tile_pool(name="persist", bufs=1) as persist,
            tc.tile_pool(name="pp", bufs=1, space="PSUM") as pp,
            tc.tile_pool(name="ps2", bufs=2, space="PSUM") as ps2,
            tc.tile_pool(name="pso", bufs=2, space="PSUM") as pso,
            tc.tile_pool(name="psd", bufs=1, space="PSUM") as psd,
            tc.tile_pool(name="et", bufs=4) as et,
            tc.tile_pool(name="ot", bufs=2) as ot,
            tc.tile_pool(name="dt", bufs=1) as dt_pool,
        ):
            # ---- resident SBUF tensors ----
            xq8_t = persist.tile([128, NDC, S], f8, tag="xq8")
            xv16_t = persist.tile([128, NDC, 2 * KC], bf, tag="xv16")
            w8_t = persist.tile([128, 3, NDC, DOUT], f8, tag="w8")
            i8_t = persist.tile([128, 128], f8, tag="i8")
            wv16_t = persist.tile([128, NDC, DOUT], bf, tag="wv16")
            mk8_t = persist.tile([128, 2, TQ], f8, tag="mk8")
            oh8_t = persist.tile([128, 2, 8 * NQB], f8, tag="oh8")
            oh16_t = persist.tile([128, 8], bf, tag="oh16")
            qT = persist.tile([128, S], bf, tag="qT")
            kT = persist.tile([128, SK], bf, tag="kT")
            v8_t = persist.tile([128, NKLOC, DOUT], f8, tag="v8")
            v16_t = persist.tile([128, 2, DOUT], bf, tag="v16")

            # ---- input DMA: 3 issue queues in parallel, first-needed pieces
            # first on each. Piece g of xq8 feeds kproj(g), vproj(g),
            # qproj(2g), qproj(2g+1).
            nc.sync.dma_start(out=w8_t[:], in_=w8_d[:])
            nc.sync.dma_start(out=i8_t[:], in_=i8_d[:])
            nc.sync.dma_start(out=xq8_t[:, :, 0:1024], in_=xq8_d[:, :, 0:1024])
            nc.scalar.dma_start(out=mk8_t[:], in_=mk8_d[:])
            nc.scalar.dma_start(out=oh8_t[:], in_=oh8_d[:])
            nc.scalar.dma_start(out=oh16_t[:], in_=oh16_d[:])
            nc.scalar.dma_start(
                out=xq8_t[:, :, 1024:2048], in_=xq8_d[:, :, 1024:2048]
            )
            nc.gpsimd.dma_start(out=xv16_t[:], in_=xv16_d[:])
            nc.gpsimd.dma_start(out=wv16_t[:], in_=wv16_d[:])
            nc.sync.dma_start(
                out=xq8_t[:, :, 2048:3072], in_=xq8_d[:, :, 2048:3072]
            )
            nc.gpsimd.dma_start(out=xq8_t[:, :, 3072:S], in_=xq8_d[:, :, 3072:S])

            # owned-key x columns: local chunk c lives at block floor(c/2),
            # column offset 128*(c%2)
            def xcol(c):
                return TQ * (c // 2) + KC * (c % 2)

            def kproj(g):  # K^T for local chunks 4g..4g+3
                ps = pp.tile([128, 512], f32, tag="pp", name=f"ppk{g}")
                xb = xq8_t.rearrange("p t (b c) -> p t b c", c=TQ)
                for j in (0, 1):
                    nc.tensor.matmul(
                        ps[:],
                        w8_t[:, 1, 2 * j:2 * j + 2, :],
                        xb[:, 2 * j:2 * j + 2, 2 * g:2 * g + 2, 0:2 * KC],
                        start=(j == 0),
                        stop=(j == 1),
                        perf_mode=DR,
                    )
                nc.vector.tensor_copy(kT[:, 512 * g:512 * (g + 1)], ps[:])

            def qproj(g):  # Q^T for permuted queries [512g, 512g+512)
                ps = pp.tile([128, 512], f32, tag="pp", name=f"ppq{g}")
                for j in (0, 1):
                    nc.tensor.matmul(
                        ps[:],
                        w8_t[:, 0, 2 * j:2 * j + 2, :],
                        xq8_t[:, 2 * j:2 * j + 2, 512 * g:512 * (g + 1)],
                        start=(j == 0),
                        stop=(j == 1),
                        perf_mode=DR,
                    )
                nc.vector.tensor_copy(qT[:, 512 * g:512 * (g + 1)], ps[:])

            def vproj(g):  # V for local chunks 4g..4g+3, fp8
                ps = pp.tile([128, 4, DOUT], f32, tag="pp", name=f"ppv{g}")
                for c in range(4):
                    ck = 4 * g + c
                    for j in (0, 1):
                        nc.tensor.matmul(
                            ps[:, c, :],
                            xq8_t[:, 2 * j:2 * j + 2, xcol(ck):xcol(ck) + KC],
                            w8_t[:, 2, 2 * j:2 * j + 2, :],
                            start=(j == 0),
                            stop=(j == 1),
                            perf_mode=DR,
                        )
                nc.vector.tensor_copy(v8_t[:, 4 * g:4 * (g + 1), :], ps[:])

            def v16proj():  # bf16 V for local chunks 0,1 (block-0 accuracy)
                ps = pp.tile([128, 2, DOUT], f32, tag="pp", name="ppv16")
                for c in (0, 1):
                    for t in range(NDC):
                        nc.tensor.matmul(
                            ps[:, c, :],
                            xv16_t[:, t, KC * c:KC * (c + 1)],
                            wv16_t[:, t, :],
                            start=(t == 0),
                            stop=(t == NDC - 1),
                        )
                nc.vector.tensor_copy(v16_t[:], ps[:])

            pd = psd.tile([8, TQ], f32, tag="pd", name="pd")

            def attn_block(qg, last_den_blk=False):
                npairs = qg + 1
                po = pso.tile([128, TQ], f32, tag="po", name=f"po{qg}")
                for i in range(npairs):
                    masked = i == npairs - 1
                    pair = ps2.tile([128, 2, TQ], f32, tag="ps2", name=f"ps{qg}_{i}")
                    for c in (0, 1):
                        ck = 2 * i + c
                        if masked:
                            # identity matmul deposits the additive causal
                            # mask into PSUM -- tensor-queue-local, no
                            # cross-engine dependency
                            nc.tensor.matmul(
                                pair[:, c, :],
                                i8_t[:],
                                mk8_t[:, c, :],
                                start=True,
                                stop=False,
                                skip_group_check=True,
                            )
                        nc.tensor.matmul(
                            pair[:, c, :],
                            kT[:, KC * ck:KC * (ck + 1)],
                            qT[:, TQ * qg:TQ * (qg + 1)],
                            start=not masked,
                            stop=True,
                            skip_group_check=masked,
                        )
                    edt = bf if qg == 0 else f8
                    etag = "e16" if qg == 0 else "e8"
                    e = et.tile([128, 2, TQ], edt, tag=etag, name=f"e{qg}_{i}")
                    nc.scalar.activation(e[:], pair[:], EXP, scale=RSQRT_D)
                    last_den = last_den_blk and i == npairs - 1
                    if qg == 0:
                        for c in (0, 1):
                            nc.tensor.matmul(
                                po[:],
                                v16_t[:, c, :],
                                e[:, c, :],
                                start=(c == 0),
                                stop=(c == 1),
                            )
                            nc.tensor.matmul(
                                pd[:],
                                oh16_t[:],
                                e[:, c, :],
                                start=(c == 0),
                                stop=False,
                                skip_group_check=True,
                            )
                    else:
                        nc.tensor.matmul(
                            po[:],
                            v8_t[:, 2 * i:2 * i + 2, :],
                            e[:],
                            start=(i == 0),
                            stop=(i == npairs - 1),
                            perf_mode=DR,
                        )
                        nc.tensor.matmul(
                            pd[:],
                            oh8_t[:, :, 8 * qg:8 * (qg + 1)],
                            e[:],
                            start=False,
                            stop=last_den,
                            perf_mode=DR,
                            skip_group_check=True,
                        )
                o = ot.tile([128, TQ], f32, tag="o", name=f"o{qg}")
                nc.vector.tensor_copy(o[:], po[:])
                nc.gpsimd.dma_start(out=numT[:, TQ * qg:TQ * (qg + 1)], in_=o[:])

            # ---- schedule: projections interleaved as tensor-engine filler;
            # big blocks early (while filler exists), small block last
            kproj(0)
            v16proj()
            vproj(0)
            qproj(0)
            attn_block(0)
            kproj(1)
            vproj(1)
            qproj(1)
            attn_block(1)
            kproj(2)
            vproj(2)
            qproj(2)
            attn_block(2)
            kproj(3)
            vproj(3)
            qproj(3)
            attn_block(3)
            qproj(7)
            attn_block(7)
            qproj(6)
            attn_block(6)
            qproj(5)
            attn_block(5)
            qproj(4)
            attn_block(4, last_den_blk=True)

            d = dt_pool.tile([8, TQ], f32, tag="d", name="d")
            nc.vector.tensor_copy(d[:], pd[:])
            nc.gpsimd.dma_start(out=den[:, :], in_=d[:])

    nc.finalize()
    return nc


def _perm(par):
    # within-block permutation: owned keys first.
    # pi[r] = original offset of permuted position r
    r = np.arange(TQ)
    return np.where(r < TQ // 2, 2 * r + par, 2 * (r - TQ // 2) + (1 - par))


def _build_masks8(par):
    # additive causal masks (0 / -240 fp8) for the last pair of each query
    # block, in PERMUTED query order: pair-half c covers owned diagonal
    # offsets 256c + 2k + par; query position r is original offset pi(r).
    pi = _perm(par)  # [512]
    c = np.arange(2)[:, None, None]
    k = np.arange(KC)[None, :, None]
    allowed = (256 * c + 2 * k + par) <= pi[None, None, :]
    return np.where(allowed, np.float32(0.0), np.float32(-240.0))  # [2, 128, 512]


def _get_nc():
    if "nc" not in _cache:
        _cache["nc"] = _build_nc()
    return _cache["nc"]


def _pack_pm(a):
    # [DIN, cols] -> partition-major [128, DIN//128, cols]
    return np.ascontiguousarray(a.reshape(DIN // 128, 128, a.shape[1]).transpose(1, 0, 2))


def _prepare_in_maps(x, Wq, Wk, Wv):
    import ml_dtypes

    f8 = ml_dtypes.float8_e4m3
    bf = ml_dtypes.bfloat16

    # [128, 3, NDC, DOUT]: w8[p, i, c, e] = W_i[128c + p, e]
    w8 = np.stack([_pack_pm(w).reshape(128, NDC, DOUT) for w in (Wq, Wk, Wv)], axis=1)
    w8 = np.ascontiguousarray(w8).astype(f8)
    wv16 = _pack_pm(Wv).astype(bf)
    i8 = np.eye(128, dtype=np.float32).astype(f8)

    # one-hot denominator routers
    oh8 = np.zeros((128, 2, 8 * NQB), dtype=np.float32)
    for qg in range(NQB):
        oh8[:, :, 8 * qg + qg] = 1.0
    oh8 = oh8.astype(f8)
    oh16 = np.zeros((128, 8), dtype=np.float32)
    oh16[:, 0] = 1.0
    oh16 = oh16.astype(bf)

    in_maps = []
    for c in range(NCORES):
        b, par = c // 2, c % 2
        pi = _perm(par)
        gidx = (np.arange(S) // TQ) * TQ  # block base per position
        xp = x[b].T.astype(np.float32)[:, gidx + pi[np.arange(S) % TQ]]
        m = _build_masks8(par)  # [2, 128, 512]
        mk8 = np.ascontiguousarray(m.transpose(1, 0, 2)).astype(f8)
        in_maps.append({
            "xq8": _pack_pm(xp).astype(f8),
            "xv16": _pack_pm(np.ascontiguousarray(xp[:, :2 * KC])).astype(bf),
            "w8": w8,
            "i8": i8,
            "wv16": wv16,
            "mk8": mk8,
            "oh8": oh8,
            "oh16": oh16,
        })
    return in_maps


def _gather(results):
    out = np.empty((B, S, DOUT), dtype=np.float32)
    base = (np.arange(S) // TQ) * TQ
    for b in range(B):
        acc_num = np.zeros((S, DOUT), dtype=np.float64)
        acc_den = np.zeros(S, dtype=np.float64)
        for par in range(2):
            r = results[2 * b + par]
            gidx = base + _perm(par)[np.arange(S) % TQ]  # permuted pos -> orig
            acc_num[gidx] += r["numT"].astype(np.float64).T
            acc_den[gidx] += r["den"].astype(np.float64).reshape(-1)
        out[b] = (acc_num / acc_den[:, None]).astype(np.float32)
    return out


def kernel(**inputs):
    from concourse.bass_utils import run_bass_kernel_spmd

    x = np.asarray(inputs["x"], dtype=np.float32)
    Wq = np.asarray(inputs["Wq"], dtype=np.float32)
    Wk = np.asarray(inputs["Wk"], dtype=np.float32)
    Wv = np.asarray(inputs["Wv"], dtype=np.float32)

    nc = _get_nc()
    in_maps = _prepare_in_maps(x, Wq, Wk, Wv)
    res = run_bass_kernel_spmd(nc, in_maps, list(range(NCORES)))
    return _gather(res.results)


# revision 17
# speedup vs baseline: 1.1358x; 1.1060x over previous
"""Causal attention kernel for 8 Trainium2 NeuronCores.

Problem: x[4, 4096, 512] @ {Wq,Wk,Wv}[512, 128] -> causal attention -> [4, 4096, 128].

Sharding: 2 cores per batch, interleaved over KEY chunks. Core c = 2b+p
(batch b, parity p) owns keys {2j+p} and computes, for every query block of
its batch, the partial softmax numerator/denominator over its keys; the host
sums the two partials and divides. Causality gives both parities identical
per-block work (exact load balance, identical SPMD program).

Within every 512-query block, the sequence axis is HOST-PERMUTED per core to
"owned keys first" (pi(r) = 2r+par for r<256, 2(r-256)+(1-par) otherwise).
Queries and keys share the axis, so one fp8 x tensor serves all three
projections: K/V projections read the leading 256 columns of each block
(the owned keys) directly via strided APs -- no separate gathered copy. The
host un-permutes the output columns.

On-device layout: scores are computed transposed, S^T[key, q]:
  - projections run in fp8e4 DoubleRow (2 contraction k-tiles per pass),
    outputs cast to bf16 (K^T, Q^T) / fp8 (V)
  - S^T chunk = bf16 matmul(lhsT=K^T[:, chunk], rhs=Q^T[:, qblock])
  - chunks are processed in PAIRS: both score matmuls land in one 2-bank
    PSUM tile; ONE ScalarE exp (scale=1/sqrt(d)) covers the pair,
    amortizing the fixed activation overhead; output E is fp8
  - the causal mask (0/-240, fp8) is written into the pair's PSUM by an
    identity matmul BEFORE the scores accumulate (start=False), keeping
    masking entirely on the tensor queue (no cross-engine stall); after
    exp the masked scores flush to exactly 0 in fp8
  - numerator: ONE fp8 DoubleRow matmul per pair (lhsT = V chunk pair)
  - denominator: ONE fp8 DoubleRow matmul per pair with a one-hot
    stationary [128, 2, 8] routing the sum into row qg of a single
    persistent PSUM bank [8, 512] holding all 8 blocks' denominators
  - query block 0 runs E/V in bf16 (early rows are near-copies of V and
    cannot tolerate fp8 quantization; everywhere else softmax averaging
    suppresses it)
  - block order 0,1,2,3,7,6,5,4: big blocks run while projection filler
    exists; the kernel drains on a small block
"""

import math

import numpy as np

B, S, DIN, DOUT = 4, 4096, 512, 128
NCORES = 8
TQ = 512            # query block size
NQB = S // TQ       # 8 query blocks per batch
KC = 128            # key chunk size
NKLOC = S // KC // 2  # 16 key chunks owned per core
SK = NKLOC * KC     # 2048 owned keys
NDC = DIN // 128    # 4 contraction chunks
RSQRT_D = 1.0 / math.sqrt(float(DOUT))

_cache = {}


def _build_nc():
    import concourse.bacc as bacc
    import concourse.mybir as mybir
    import concourse.tile as tile

    f32 = mybir.dt.float32
    bf = mybir.dt.bfloat16
    f8 = mybir.dt.float8e4
    DR = mybir.MatmulPerfMode.DoubleRow
    EXP = mybir.ActivationFunctionType.Exp

    nc = bacc.Bacc(None, target_bir_lowering=False, debug=False)

    # ---- DRAM parameters ----
    xq8_d = nc.declare_dram_parameter("xq8", [128, NDC, S], f8, isOutput=False)
    xv16_d = nc.declare_dram_parameter("xv16", [128, NDC, 2 * KC], bf, isOutput=False)
    w8_d = nc.declare_dram_parameter("w8", [128, 3, NDC, DOUT], f8, isOutput=False)
    i8_d = nc.declare_dram_parameter("i8", [128, 128], f8, isOutput=False)
    wv16_d = nc.declare_dram_parameter("wv16", [128, NDC, DOUT], bf, isOutput=False)
    mk8_d = nc.declare_dram_parameter("mk8", [128, 2, TQ], f8, isOutput=False)
    oh8_d = nc.declare_dram_parameter("oh8", [128, 2, 8 * NQB], f8, isOutput=False)
    oh16_d = nc.declare_dram_parameter("oh16", [128, 8], bf, isOutput=False)
    numT = nc.declare_dram_parameter("numT", [DOUT, S], f32, isOutput=True)
    den = nc.declare_dram_parameter("den", [NQB, TQ], f32, isOutput=True)

    with tile.TileContext(nc) as tc:
        with (
            tc.tile_pool(name="persist", bufs=1) as persist,
            tc.tile_pool(name="pp", bufs=1, space="PSUM") as pp,
            tc.tile_pool(name="ps2", bufs=2, space="PSUM") as ps2,
            tc.tile_pool(name="pso", bufs=2, space="PSUM") as pso,
            tc.tile_pool(name="psd", bufs=1, space="PSUM") as psd,
            tc.tile_pool(name="et", bufs=4) as et,
            tc.tile_pool(name="ot", bufs=2) as ot,
            tc.tile_pool(name="dt", bufs=1) as dt_pool,
        ):
            # ---- resident SBUF tensors ----
            xq8_t = persist.tile([128, NDC, S], f8, tag="xq8")
            xv16_t = persist.tile([128, NDC, 2 * KC], bf, tag="xv16")
            w8_t = persist.tile([128, 3, NDC, DOUT], f8, tag="w8")
            i8_t = persist.tile([128, 128], f8, tag="i8")
            wv16_t = persist.tile([128, NDC, DOUT], bf, tag="wv16")
            mk8_t = persist.tile([128, 2, TQ], f8, tag="mk8")
            oh8_t = persist.tile([128, 2, 8 * NQB], f8, tag="oh8")
            oh16_t = persist.tile([128, 8], bf, tag="oh16")
            qT = persist.tile([128, S], bf, tag="qT")
            kT = persist.tile([128, SK], bf, tag="kT")
            v8_t = persist.tile([128, NKLOC, DOUT], f8, tag="v8")
            v16_t = persist.tile([128, 2, DOUT], bf, tag="v16")

            # ---- input DMA: per-dma_start transfer rate is ~45-50 GB/s and
            # transfers on one issue queue serialize, so spread pieces over
            # all 3 queues, ordered by first-need (greedy by deadline).
            def xpiece(eng, a, b):
                eng.dma_start(out=xq8_t[:, :, a:b], in_=xq8_d[:, :, a:b])

            nc.sync.dma_start(out=w8_t[:, 1], in_=w8_d[:, 1])      # Wk
            nc.scalar.dma_start(out=w8_t[:, 0], in_=w8_d[:, 0])    # Wq
            nc.gpsimd.dma_start(out=w8_t[:, 2], in_=w8_d[:, 2])    # Wv
            nc.gpsimd.dma_start(out=i8_t[:], in_=i8_d[:])
            xpiece(nc.sync, 0, 512)        # A: blk0 owned + queries blk0
            xpiece(nc.scalar, 512, 768)    # B: blk1 owned (kproj0)
            nc.scalar.dma_start(out=mk8_t[:], in_=mk8_d[:])
            nc.scalar.dma_start(out=oh8_t[:], in_=oh8_d[:])
            nc.scalar.dma_start(out=oh16_t[:], in_=oh16_d[:])
            nc.gpsimd.dma_start(out=xv16_t[:], in_=xv16_d[:])
            nc.gpsimd.dma_start(out=wv16_t[:], in_=wv16_d[:])
            xpiece(nc.gpsimd, 768, 1024)   # B2: queries blk1
            xpiece(nc.sync, 1024, 1536)    # C: blk2 (kproj1, qproj2)
            xpiece(nc.scalar, 1536, 2048)  # D: blk3 (kproj1, qproj3)
            xpiece(nc.gpsimd, 2560, 3072)  # F: blk5 (kproj2, v10/11, qproj5)
            xpiece(nc.sync, 2048, 2560)    # E: blk4 (kproj2, v8/9, qproj4)
            xpiece(nc.scalar, 3584, S)     # H: blk7 (qproj7, kproj3)
            xpiece(nc.sync, 3072, 3584)    # G: blk6 (kproj3, qproj6)

            # owned-key x columns: local chunk c lives at block floor(c/2),
            # column offset 128*(c%2)
            def xcol(c):
                return TQ * (c // 2) + KC * (c % 2)

            def kproj(g):  # K^T for local chunks 4g..4g+3
                ps = pp.tile([128, 512], f32, tag="pp", name=f"ppk{g}")
                xb = xq8_t.rearrange("p t (b c) -> p t b c", c=TQ)
                for j in (0, 1):
                    nc.tensor.matmul(
                        ps[:],
                        w8_t[:, 1, 2 * j:2 * j + 2, :],
                        xb[:, 2 * j:2 * j + 2, 2 * g:2 * g + 2, 0:2 * KC],
                        start=(j == 0),
                        stop=(j == 1),
                        perf_mode=DR,
                    )
                nc.vector.tensor_copy(kT[:, 512 * g:512 * (g + 1)], ps[:])

            def qproj(g):  # Q^T for permuted queries [512g, 512g+512)
                ps = pp.tile([128, 512], f32, tag="pp", name=f"ppq{g}")
                for j in (0, 1):
                    nc.tensor.matmul(
                        ps[:],
                        w8_t[:, 0, 2 * j:2 * j + 2, :],
                        xq8_t[:, 2 * j:2 * j + 2, 512 * g:512 * (g + 1)],
                        start=(j == 0),
                        stop=(j == 1),
                        perf_mode=DR,
                    )
                nc.vector.tensor_copy(qT[:, 512 * g:512 * (g + 1)], ps[:])

            def vproj2(g):  # V for local chunks 2g, 2g+1, fp8
                ps = pp.tile([128, 2, DOUT], f32, tag="pp", name=f"ppv{g}")
                for c in (0, 1):
                    ck = 2 * g + c
                    for j in (0, 1):
                        nc.tensor.matmul(
                            ps[:, c, :],
                            xq8_t[:, 2 * j:2 * j + 2, xcol(ck):xcol(ck) + KC],
                            w8_t[:, 2, 2 * j:2 * j + 2, :],
                            start=(j == 0),
                            stop=(j == 1),
                            perf_mode=DR,
                        )
                nc.vector.tensor_copy(v8_t[:, 2 * g:2 * (g + 1), :], ps[:])

            def v16proj():  # bf16 V for local chunks 0,1 (block-0 accuracy)
                ps = pp.tile([128, 2, DOUT], f32, tag="pp", name="ppv16")
                for c in (0, 1):
                    for t in range(NDC):
                        nc.tensor.matmul(
                            ps[:, c, :],
                            xv16_t[:, t, KC * c:KC * (c + 1)],
                            wv16_t[:, t, :],
                            start=(t == 0),
                            stop=(t == NDC - 1),
                        )
                nc.vector.tensor_copy(v16_t[:], ps[:])

            pd = psd.tile([8, TQ], f32, tag="pd", name="pd")

            def attn_block(qg, filler, last_den_blk=False):
                npairs = qg + 1
                po = pso.tile([128, TQ], f32, tag="po", name=f"po{qg}")
                for i in range(npairs):
                    # one projection filler unit per pair keeps the tensor
                    # queue fed while exp runs (p-state stays ramped)
                    if filler:
                        filler.pop(0)()
                    masked = i == npairs - 1
                    pair = ps2.tile([128, 2, TQ], f32, tag="ps2", name=f"ps{qg}_{i}")
                    for c in (0, 1):
                        ck = 2 * i + c
                        if masked:
                            # identity matmul deposits the additive causal
                            # mask into PSUM -- tensor-queue-local, no
                            # cross-engine dependency
                            nc.tensor.matmul(
                                pair[:, c, :],
                                i8_t[:],
                                mk8_t[:, c, :],
                                start=True,
                                stop=False,
                                skip_group_check=True,
                            )
                        nc.tensor.matmul(
                            pair[:, c, :],
                            kT[:, KC * ck:KC * (ck + 1)],
                            qT[:, TQ * qg:TQ * (qg + 1)],
                            start=not masked,
                            stop=True,
                            skip_group_check=masked,
                        )
                    edt = bf if qg == 0 else f8
                    etag = "e16" if qg == 0 else "e8"
                    e = et.tile([128, 2, TQ], edt, tag=etag, name=f"e{qg}_{i}")
                    nc.scalar.activation(e[:], pair[:], EXP, scale=RSQRT_D)
                    last_den = last_den_blk and i == npairs - 1
                    if qg == 0:
                        for c in (0, 1):
                            nc.tensor.matmul(
                                po[:],
                                v16_t[:, c, :],
                                e[:, c, :],
                                start=(c == 0),
                                stop=(c == 1),
                            )
                            nc.tensor.matmul(
                                pd[:],
                                oh16_t[:],
                                e[:, c, :],
                                start=(c == 0),
                                stop=False,
                                skip_group_check=True,
                            )
                    else:
                        nc.tensor.matmul(
                            po[:],
                            v8_t[:, 2 * i:2 * i + 2, :],
                            e[:],
                            start=(i == 0),
                            stop=(i == npairs - 1),
                            perf_mode=DR,
                        )
                        nc.tensor.matmul(
                            pd[:],
                            oh8_t[:, :, 8 * qg:8 * (qg + 1)],
                            e[:],
                            start=False,
                            stop=last_den,
                            perf_mode=DR,
                            skip_group_check=True,
                        )
                o = ot.tile([128, TQ], f32, tag="o", name=f"o{qg}")
                nc.vector.tensor_copy(o[:], po[:])
                nc.gpsimd.dma_start(out=numT[:, TQ * qg:TQ * (qg + 1)], in_=o[:])

            # ---- schedule: upfront projections for block 0, then big blocks
            # early / small block last, with the remaining projection work
            # drip-fed one unit per attention pair (deadline-ordered)
            kproj(0)
            v16proj()
            vproj2(0)
            vproj2(1)
            qproj(0)
            filler = [
                lambda: qproj(1),
                lambda: qproj(2),
                lambda: kproj(1),
                lambda: vproj2(2),
                lambda: qproj(3),
                lambda: vproj2(3),
                lambda: qproj(7),
                lambda: kproj(2),
                lambda: vproj2(4),
                lambda: vproj2(5),
                lambda: kproj(3),
                lambda: vproj2(6),
                lambda: vproj2(7),
                lambda: qproj(6),
                lambda: qproj(5),
                lambda: qproj(4),
            ]
            attn_block(0, filler)
            attn_block(1, filler)
            attn_block(2, filler)
            attn_block(3, filler)
            attn_block(7, filler)
            attn_block(6, filler)
            attn_block(5, filler)
            attn_block(4, filler, last_den_blk=True)

            d = dt_pool.tile([8, TQ], f32, tag="d", name="d")
            nc.vector.tensor_copy(d[:], pd[:])
            nc.gpsimd.dma_start(out=den[:, :], in_=d[:])

    nc.finalize()
    return nc


def _perm(par):
    # within-block permutation: owned keys first.
    # pi[r] = original offset of permuted position r
    r = np.arange(TQ)
    return np.where(r < TQ // 2, 2 * r + par, 2 * (r - TQ // 2) + (1 - par))


def _build_masks8(par):
    # additive causal masks (0 / -240 fp8) for the last pair of each query
    # block, in PERMUTED query order: pair-half c covers owned diagonal
    # offsets 256c + 2k + par; query position r is original offset pi(r).
    pi = _perm(par)  # [512]
    c = np.arange(2)[:, None, None]
    k = np.arange(KC)[None, :, None]
    allowed = (256 * c + 2 * k + par) <= pi[None, None, :]
    return np.where(allowed, np.float32(0.0), np.float32(-240.0))  # [2, 128, 512]


def _get_nc():
    if "nc" not in _cache:
        _cache["nc"] = _build_nc()
    return _cache["nc"]


def _pack_pm(a):
    # [DIN, cols] -> partition-major [128, DIN//128, cols]
    return np.ascontiguousarray(a.reshape(DIN // 128, 128, a.shape[1]).transpose(1, 0, 2))


def _prepare_in_maps(x, Wq, Wk, Wv):
    import ml_dtypes

    f8 = ml_dtypes.float8_e4m3
    bf = ml_dtypes.bfloat16

    # [128, 3, NDC, DOUT]: w8[p, i, c, e] = W_i[128c + p, e]
    w8 = np.stack([_pack_pm(w).reshape(128, NDC, DOUT) for w in (Wq, Wk, Wv)], axis=1)
    w8 = np.ascontiguousarray(w8).astype(f8)
    wv16 = _pack_pm(Wv).astype(bf)
    i8 = np.eye(128, dtype=np.float32).astype(f8)

    # one-hot denominator routers
    oh8 = np.zeros((128, 2, 8 * NQB), dtype=np.float32)
    for qg in range(NQB):
        oh8[:, :, 8 * qg + qg] = 1.0
    oh8 = oh8.astype(f8)
    oh16 = np.zeros((128, 8), dtype=np.float32)
    oh16[:, 0] = 1.0
    oh16 = oh16.astype(bf)

    in_maps = []
    for c in range(NCORES):
        b, par = c // 2, c % 2
        pi = _perm(par)
        gidx = (np.arange(S) // TQ) * TQ  # block base per position
        xp = x[b].T.astype(np.float32)[:, gidx + pi[np.arange(S) % TQ]]
        m = _build_masks8(par)  # [2, 128, 512]
        mk8 = np.ascontiguousarray(m.transpose(1, 0, 2)).astype(f8)
        in_maps.append({
            "xq8": _pack_pm(xp).astype(f8),
            "xv16": _pack_pm(np.ascontiguousarray(xp[:, :2 * KC])).astype(bf),
            "w8": w8,
            "i8": i8,
            "wv16": wv16,
            "mk8": mk8,
            "oh8": oh8,
            "oh16": oh16,
        })
    return in_maps


def _gather(results):
    out = np.empty((B, S, DOUT), dtype=np.float32)
    base = (np.arange(S) // TQ) * TQ
    for b in range(B):
        acc_num = np.zeros((S, DOUT), dtype=np.float64)
        acc_den = np.zeros(S, dtype=np.float64)
        for par in range(2):
            r = results[2 * b + par]
            gidx = base + _perm(par)[np.arange(S) % TQ]  # permuted pos -> orig
            acc_num[gidx] += r["numT"].astype(np.float64).T
            acc_den[gidx] += r["den"].astype(np.float64).reshape(-1)
        out[b] = (acc_num / acc_den[:, None]).astype(np.float32)
    return out


def kernel(**inputs):
    from concourse.bass_utils import run_bass_kernel_spmd

    x = np.asarray(inputs["x"], dtype=np.float32)
    Wq = np.asarray(inputs["Wq"], dtype=np.float32)
    Wk = np.asarray(inputs["Wk"], dtype=np.float32)
    Wv = np.asarray(inputs["Wv"], dtype=np.float32)

    nc = _get_nc()
    in_maps = _prepare_in_maps(x, Wq, Wk, Wv)
    res = run_bass_kernel_spmd(nc, in_maps, list(range(NCORES)))
    return _gather(res.results)


# revision 19
# speedup vs baseline: 1.1893x; 1.0471x over previous
"""Causal attention kernel for 8 Trainium2 NeuronCores.

Problem: x[4, 4096, 512] @ {Wq,Wk,Wv}[512, 128] -> causal attention -> [4, 4096, 128].

Sharding: 2 cores per batch, interleaved over KEY chunks. Core c = 2b+p
(batch b, parity p) owns keys {2j+p} and computes, for every query block of
its batch, the partial softmax numerator/denominator over its keys; the host
sums the two partials and divides. Causality gives both parities identical
per-block work (exact load balance, identical SPMD program).

Within every 512-query block, the sequence axis is HOST-PERMUTED per core to
"owned keys first" (pi(r) = 2r+par for r<256, 2(r-256)+(1-par) otherwise).
Queries and keys share the axis, so one fp8 x tensor serves all three
projections: K/V projections read the leading 256 columns of each block
(the owned keys) directly via strided APs -- no separate gathered copy. The
host un-permutes the output columns.

On-device layout: scores are computed transposed, S^T[key, q]:
  - projections run in fp8e4 DoubleRow (2 contraction k-tiles per pass),
    outputs cast to bf16 (K^T, Q^T) / fp8 (V)
  - S^T chunk = bf16 matmul(lhsT=K^T[:, chunk], rhs=Q^T[:, qblock])
  - chunks are processed in PAIRS: both score matmuls land in one 2-bank
    PSUM tile; ONE ScalarE exp (scale=1/sqrt(d)) covers the pair,
    amortizing the fixed activation overhead; output E is fp8
  - the causal mask (0/-240) is BUILT ON DEVICE during the DMA-bound
    startup (pi row-vector broadcast via a K=1 matmul, two fused DVE
    tensor_scalar ops) and deposited into the masked pair's PSUM by an
    identity matmul before the scores accumulate -- masking never leaves
    the tensor queue, and exp flushes masked scores to exactly 0 in fp8
  - numerator: ONE fp8 DoubleRow matmul per pair (lhsT = V chunk pair)
  - denominator: ONE fp8 DoubleRow matmul per pair with a one-hot
    stationary [128, 2, 8] routing the sum into row qg of a single
    persistent PSUM bank [8, 512] holding all 8 blocks' denominators
  - query block 0 runs E/V in bf16 (early rows are near-copies of V and
    cannot tolerate fp8 quantization; elsewhere softmax averaging
    suppresses it)
  - a short warmup matmul chain runs while the first DMA pieces land so
    the PE p-state is ramped when real work arrives; projection work is
    split into ~17 filler units drip-fed one per attention pair to keep
    the tensor queue continuously busy; big blocks run early, a small
    block drains the kernel
"""

import math

import numpy as np

B, S, DIN, DOUT = 4, 4096, 512, 128
NCORES = 8
TQ = 512            # query block size
NQB = S // TQ       # 8 query blocks per batch
KC = 128            # key chunk size
NKLOC = S // KC // 2  # 16 key chunks owned per core
SK = NKLOC * KC     # 2048 owned keys
NDC = DIN // 128    # 4 contraction chunks
RSQRT_D = 1.0 / math.sqrt(float(DOUT))
NWARM = 12

_cache = {}


def _build_nc():
    import concourse.bacc as bacc
    import concourse.mybir as mybir
    import concourse.tile as tile

    f32 = mybir.dt.float32
    bf = mybir.dt.bfloat16
    f8 = mybir.dt.float8e4
    DR = mybir.MatmulPerfMode.DoubleRow
    EXP = mybir.ActivationFunctionType.Exp
    ALU = mybir.AluOpType

    nc = bacc.Bacc(None, target_bir_lowering=False, debug=False)

    # ---- DRAM parameters ----
    xq8_d = nc.declare_dram_parameter("xq8", [128, NDC, S], f8, isOutput=False)
    xv16_d = nc.declare_dram_parameter("xv16", [128, NDC, 2 * KC], bf, isOutput=False)
    w8_d = nc.declare_dram_parameter("w8", [128, 3, NDC, DOUT], f8, isOutput=False)
    i8_d = nc.declare_dram_parameter("i8", [128, 128], f8, isOutput=False)
    wv16_d = nc.declare_dram_parameter("wv16", [128, NDC, DOUT], bf, isOutput=False)
    pib_d = nc.declare_dram_parameter("pib", [1, TQ], bf, isOutput=False)
    koff_d = nc.declare_dram_parameter("koff", [128, 2], f32, isOutput=False)
    oh8_d = nc.declare_dram_parameter("oh8", [128, 2, 8 * NQB], f8, isOutput=False)
    oh16_d = nc.declare_dram_parameter("oh16", [128, 8], bf, isOutput=False)
    numT = nc.declare_dram_parameter("numT", [DOUT, S], f32, isOutput=True)
    den = nc.declare_dram_parameter("den", [NQB, TQ], f32, isOutput=True)

    with tile.TileContext(nc) as tc:
        with (
            tc.tile_pool(name="persist", bufs=1) as persist,
            tc.tile_pool(name="pp", bufs=2, space="PSUM") as pp,
            tc.tile_pool(name="ps2", bufs=2, space="PSUM") as ps2,
            tc.tile_pool(name="pso", bufs=1, space="PSUM") as pso,
            tc.tile_pool(name="psd", bufs=1, space="PSUM") as psd,
            tc.tile_pool(name="et", bufs=4) as et,
            tc.tile_pool(name="ot", bufs=2) as ot,
            tc.tile_pool(name="dt", bufs=1) as dt_pool,
        ):
            # ---- resident SBUF tensors ----
            xq8_t = persist.tile([128, NDC, S], f8, tag="xq8")
            xv16_t = persist.tile([128, NDC, 2 * KC], bf, tag="xv16")
            w8_t = persist.tile([128, 3, NDC, DOUT], f8, tag="w8")
            i8_t = persist.tile([128, 128], f8, tag="i8")
            wv16_t = persist.tile([128, NDC, DOUT], bf, tag="wv16")
            pib_t = persist.tile([1, TQ], bf, tag="pib")
            koff_t = persist.tile([128, 2], f32, tag="koff")
            mk8_t = persist.tile([128, 2, TQ], f8, tag="mk8")
            oh8_t = persist.tile([128, 2, 8 * NQB], f8, tag="oh8")
            oh16_t = persist.tile([128, 8], bf, tag="oh16")
            ones_r = persist.tile([1, 128], bf, tag="ones_r")
            mtmp = persist.tile([128, TQ], f32, tag="mtmp")
            qT = persist.tile([128, S], bf, tag="qT")
            kT = persist.tile([128, SK], bf, tag="kT")
            v8_t = persist.tile([128, NKLOC, DOUT], f8, tag="v8")
            v16_t = persist.tile([128, 2, DOUT], bf, tag="v16")

            # ---- input DMA: per-dma_start transfer rate is ~20-60 GB/s and
            # transfers on one issue queue serialize, so spread pieces over
            # all 3 queues, ordered by first-need (greedy by deadline).
            def xpiece(eng, a, b):
                eng.dma_start(out=xq8_t[:, :, a:b], in_=xq8_d[:, :, a:b])

            nc.sync.dma_start(out=i8_t[:], in_=i8_d[:])
            nc.sync.dma_start(out=w8_t[:, 1], in_=w8_d[:, 1])      # Wk
            nc.scalar.dma_start(out=pib_t[:], in_=pib_d[:])
            nc.scalar.dma_start(out=koff_t[:], in_=koff_d[:])
            nc.scalar.dma_start(out=w8_t[:, 0], in_=w8_d[:, 0])    # Wq
            nc.gpsimd.dma_start(out=w8_t[:, 2], in_=w8_d[:, 2])    # Wv
            xpiece(nc.sync, 0, 512)        # A: blk0 owned + queries blk0
            xpiece(nc.scalar, 512, 768)    # B: blk1 owned (kproj0)
            nc.scalar.dma_start(out=oh8_t[:], in_=oh8_d[:])
            nc.scalar.dma_start(out=oh16_t[:], in_=oh16_d[:])
            nc.gpsimd.dma_start(out=xv16_t[:], in_=xv16_d[:])
            nc.gpsimd.dma_start(out=wv16_t[:], in_=wv16_d[:])
            xpiece(nc.gpsimd, 768, 1024)   # B2: queries blk1
            xpiece(nc.sync, 1024, 1536)    # C: blk2 (kproj1, qproj2)
            xpiece(nc.scalar, 1536, 2048)  # D: blk3 (kproj1, qproj3)
            xpiece(nc.gpsimd, 2560, 3072)  # F: blk5 (kproj2, v10/11, qproj5)
            xpiece(nc.sync, 2048, 2560)    # E: blk4 (kproj2, v8/9, qproj4)
            xpiece(nc.scalar, 3584, S)     # H: blk7 (qproj7, kproj3)
            xpiece(nc.sync, 3072, 3584)    # G: blk6 (kproj3, qproj6)

            # ---- PE warmup: keep the tensor engine busy while the first
            # DMA pieces land so the p-state ramp is underway before real
            # work (results are never read)
            nc.vector.memset(ones_r[:], 1.0)
            warm = pso.tile([128, 128], f32, tag="po", name="warm")
            for w in range(NWARM):
                nc.tensor.matmul(
                    warm[:], i8_t[:], i8_t[:], start=True, stop=True,
                    skip_group_check=True,
                )

            # ---- on-device causal mask build (runs during DMA-bound start):
            # broadcast pi (biased by -256, bf16-exact) across partitions via
            # a K=1 matmul, then two fused DVE tensor_scalar ops per half:
            # mk8 = max(min(pi - koff, 0) * 240, -240) in {0, -240}
            pib_ps = psd.tile([128, TQ], f32, tag="pd", name="pib_ps")
            nc.tensor.matmul(pib_ps[:], ones_r[:], pib_t[:], start=True, stop=True)
            for c in (0, 1):
                nc.vector.tensor_scalar(
                    mtmp[:], pib_ps[:], koff_t[:, c:c + 1], 0.0,
                    op0=ALU.subtract, op1=ALU.min,
                )
                nc.vector.tensor_scalar(
                    mk8_t[:, c, :], mtmp[:], 240.0, -240.0,
                    op0=ALU.mult, op1=ALU.max,
                )

            # owned-key x columns: local chunk c lives at block floor(c/2),
            # column offset 128*(c%2)
            def xcol(c):
                return TQ * (c // 2) + KC * (c % 2)

            def kproj(g):  # K^T for local chunks 4g..4g+3
                ps = pp.tile([128, 512], f32, tag="pp", name=f"ppk{g}")
                xb = xq8_t.rearrange("p t (b c) -> p t b c", c=TQ)
                for j in (0, 1):
                    nc.tensor.matmul(
                        ps[:],
                        w8_t[:, 1, 2 * j:2 * j + 2, :],
                        xb[:, 2 * j:2 * j + 2, 2 * g:2 * g + 2, 0:2 * KC],
                        start=(j == 0),
                        stop=(j == 1),
                        perf_mode=DR,
                    )
                nc.vector.tensor_copy(kT[:, 512 * g:512 * (g + 1)], ps[:])

            def qproj(g):  # Q^T for permuted queries [512g, 512g+512)
                ps = pp.tile([128, 512], f32, tag="pp", name=f"ppq{g}")
                for j in (0, 1):
                    nc.tensor.matmul(
                        ps[:],
                        w8_t[:, 0, 2 * j:2 * j + 2, :],
                        xq8_t[:, 2 * j:2 * j + 2, 512 * g:512 * (g + 1)],
                        start=(j == 0),
                        stop=(j == 1),
                        perf_mode=DR,
                    )
                nc.vector.tensor_copy(qT[:, 512 * g:512 * (g + 1)], ps[:])

            def vproj2(g):  # V for local chunks 2g, 2g+1, fp8
                ps = pp.tile([128, 2, DOUT], f32, tag="pp", name=f"ppv{g}")
                for c in (0, 1):
                    ck = 2 * g + c
                    for j in (0, 1):
                        nc.tensor.matmul(
                            ps[:, c, :],
                            xq8_t[:, 2 * j:2 * j + 2, xcol(ck):xcol(ck) + KC],
                            w8_t[:, 2, 2 * j:2 * j + 2, :],
                            start=(j == 0),
                            stop=(j == 1),
                            perf_mode=DR,
                        )
                nc.vector.tensor_copy(v8_t[:, 2 * g:2 * (g + 1), :], ps[:])

            def v16proj():  # bf16 V for local chunks 0,1 (block-0 accuracy)
                ps = pp.tile([128, 2, DOUT], f32, tag="pp", name="ppv16")
                for c in (0, 1):
                    for t in range(NDC):
                        nc.tensor.matmul(
                            ps[:, c, :],
                            xv16_t[:, t, KC * c:KC * (c + 1)],
                            wv16_t[:, t, :],
                            start=(t == 0),
                            stop=(t == NDC - 1),
                        )
                nc.vector.tensor_copy(v16_t[:], ps[:])

            pd = psd.tile([8, TQ], f32, tag="pd", name="pd")

            def attn_block(qg, filler, out_engines, last_den_blk=False):
                npairs = qg + 1
                po = pso.tile([128, TQ], f32, tag="po", name=f"po{qg}")
                for i in range(npairs):
                    # one projection filler unit per pair keeps the tensor
                    # queue fed while exp runs (p-state stays ramped)
                    if filler:
                        filler.pop(0)()
                    masked = i == npairs - 1
                    pair = ps2.tile([128, 2, TQ], f32, tag="ps2", name=f"ps{qg}_{i}")
                    for c in (0, 1):
                        ck = 2 * i + c
                        if masked:
                            nc.tensor.matmul(
                                pair[:, c, :],
                                i8_t[:],
                                mk8_t[:, c, :],
                                start=True,
                                stop=False,
                                skip_group_check=True,
                            )
                        nc.tensor.matmul(
                            pair[:, c, :],
                            kT[:, KC * ck:KC * (ck + 1)],
                            qT[:, TQ * qg:TQ * (qg + 1)],
                            start=not masked,
                            stop=True,
                            skip_group_check=masked,
                        )
                    edt = bf if qg == 0 else f8
                    etag = "e16" if qg == 0 else "e8"
                    e = et.tile([128, 2, TQ], edt, tag=etag, name=f"e{qg}_{i}")
                    nc.scalar.activation(e[:], pair[:], EXP, scale=RSQRT_D)
                    last_den = last_den_blk and i == npairs - 1
                    if qg == 0:
                        for c in (0, 1):
                            nc.tensor.matmul(
                                po[:],
                                v16_t[:, c, :],
                                e[:, c, :],
                                start=(c == 0),
                                stop=(c == 1),
                            )
                            nc.tensor.matmul(
                                pd[:],
                                oh16_t[:],
                                e[:, c, :],
                                start=(c == 0),
                                stop=False,
                                skip_group_check=True,
                            )
                    else:
                        nc.tensor.matmul(
                            po[:],
                            v8_t[:, 2 * i:2 * i + 2, :],
                            e[:],
                            start=(i == 0),
                            stop=(i == npairs - 1),
                            perf_mode=DR,
                        )
                        nc.tensor.matmul(
                            pd[:],
                            oh8_t[:, :, 8 * qg:8 * (qg + 1)],
                            e[:],
                            start=False,
                            stop=last_den,
                            perf_mode=DR,
                            skip_group_check=True,
                        )
                o = ot.tile([128, TQ], f32, tag="o", name=f"o{qg}")
                nc.vector.tensor_copy(o[:], po[:])
                # split the output DMA across the given (idle) issue queues
                # so the drain of the final blocks doesn't serialize on one
                nsp = len(out_engines)
                w = TQ // nsp
                for s_i, eng in enumerate(out_engines):
                    eng.dma_start(
                        out=numT[:, TQ * qg + s_i * w:TQ * qg + (s_i + 1) * w],
                        in_=o[:, s_i * w:(s_i + 1) * w],
                    )

            # ---- schedule: upfront projections for block 0, then big blocks
            # early / small block last, with the remaining projection work
            # drip-fed one unit per attention pair (deadline-ordered)
            kproj(0)
            qproj(0)
            vproj2(0)
            vproj2(1)
            filler = [
                v16proj,
                lambda: qproj(1),
                lambda: qproj(2),
                lambda: kproj(1),
                lambda: vproj2(2),
                lambda: qproj(3),
                lambda: vproj2(3),
                lambda: qproj(7),
                lambda: kproj(2),
                lambda: vproj2(4),
                lambda: vproj2(5),
                lambda: kproj(3),
                lambda: vproj2(6),
                lambda: vproj2(7),
                lambda: qproj(6),
                lambda: qproj(5),
                lambda: qproj(4),
            ]
            gp, sy, sc = nc.gpsimd, nc.sync, nc.scalar
            attn_block(0, filler, [gp])
            attn_block(1, filler, [gp])
            attn_block(2, filler, [gp])
            attn_block(3, filler, [gp])
            attn_block(7, filler, [gp])
            attn_block(6, filler, [gp])
            attn_block(5, filler, [sy])
            attn_block(4, filler, [sy, sc], last_den_blk=True)

            d = dt_pool.tile([8, TQ], f32, tag="d", name="d")
            nc.vector.tensor_copy(d[:], pd[:])
            nc.gpsimd.dma_start(out=den[:, :], in_=d[:])

    nc.finalize()
    return nc


def _perm(par):
    # within-block permutation: owned keys first.
    # pi[r] = original offset of permuted position r
    r = np.arange(TQ)
    return np.where(r < TQ // 2, 2 * r + par, 2 * (r - TQ // 2) + (1 - par))


def _get_nc():
    if "nc" not in _cache:
        _cache["nc"] = _build_nc()
    return _cache["nc"]


def _pack_pm(a):
    # [DIN, cols] -> partition-major [128, DIN//128, cols]
    return np.ascontiguousarray(a.reshape(DIN // 128, 128, a.shape[1]).transpose(1, 0, 2))


def _prepare_in_maps(x, Wq, Wk, Wv):
    import ml_dtypes

    f8 = ml_dtypes.float8_e4m3
    bf = ml_dtypes.bfloat16

    # [128, 3, NDC, DOUT]: w8[p, i, c, e] = W_i[128c + p, e]
    w8 = np.stack([_pack_pm(w).reshape(128, NDC, DOUT) for w in (Wq, Wk, Wv)], axis=1)
    w8 = np.ascontiguousarray(w8).astype(f8)
    wv16 = _pack_pm(Wv).astype(bf)
    i8 = np.eye(128, dtype=np.float32).astype(f8)

    # one-hot denominator routers
    oh8 = np.zeros((128, 2, 8 * NQB), dtype=np.float32)
    for qg in range(NQB):
        oh8[:, :, 8 * qg + qg] = 1.0
    oh8 = oh8.astype(f8)
    oh16 = np.zeros((128, 8), dtype=np.float32)
    oh16[:, 0] = 1.0
    oh16 = oh16.astype(bf)

    in_maps = []
    for c in range(NCORES):
        b, par = c // 2, c % 2
        pi = _perm(par)
        base = (np.arange(S) // TQ) * TQ
        xp = x[b].T.astype(np.float32)[:, base + pi[np.arange(S) % TQ]]
        # mask build operands: pib = pi - 256 (bf16-exact); koff[p, c] =
        # 256c + 2p + par - 256
        pib = (pi.astype(np.float32) - 256.0)[None, :].astype(bf)
        p_idx = np.arange(128, dtype=np.float32)
        koff = np.stack(
            [256.0 * cc + 2.0 * p_idx + par - 256.0 for cc in (0, 1)], axis=1
        ).astype(np.float32)
        in_maps.append({
            "xq8": _pack_pm(xp).astype(f8),
            "xv16": _pack_pm(np.ascontiguousarray(xp[:, :2 * KC])).astype(bf),
            "w8": w8,
            "i8": i8,
            "wv16": wv16,
            "pib": pib,
            "koff": koff,
            "oh8": oh8,
            "oh16": oh16,
        })
    return in_maps


def _gather(results):
    out = np.empty((B, S, DOUT), dtype=np.float32)
    base = (np.arange(S) // TQ) * TQ
    for b in range(B):
        acc_num = np.zeros((S, DOUT), dtype=np.float64)
        acc_den = np.zeros(S, dtype=np.float64)
        for par in range(2):
            r = results[2 * b + par]
            gidx = base + _perm(par)[np.arange(S) % TQ]  # permuted pos -> orig
            acc_num[gidx] += r["numT"].astype(np.float64).T
            acc_den[gidx] += r["den"].astype(np.float64).reshape(-1)
        out[b] = (acc_num / acc_den[:, None]).astype(np.float32)
    return out


def kernel(**inputs):
    from concourse.bass_utils import run_bass_kernel_spmd

    x = np.asarray(inputs["x"], dtype=np.float32)
    Wq = np.asarray(inputs["Wq"], dtype=np.float32)
    Wk = np.asarray(inputs["Wk"], dtype=np.float32)
    Wv = np.asarray(inputs["Wv"], dtype=np.float32)

    nc = _get_nc()
    in_maps = _prepare_in_maps(x, Wq, Wk, Wv)
    res = run_bass_kernel_spmd(nc, in_maps, list(range(NCORES)))
    return _gather(res.results)


# revision 27
# speedup vs baseline: 1.2155x; 1.0220x over previous
"""Causal attention kernel for 8 Trainium2 NeuronCores.

Problem: x[4, 4096, 512] @ {Wq,Wk,Wv}[512, 128] -> causal attention -> [4, 4096, 128].

Sharding: 2 cores per batch, interleaved over KEY chunks. Core c = 2b+p
(batch b, parity p) owns keys {2j+p} and computes, for every query block of
its batch, the partial softmax numerator/denominator over its keys; the host
sums the two partials and divides. Causality gives both parities identical
per-block work (exact load balance, identical SPMD program).

Within every 512-query block, the sequence axis is HOST-PERMUTED per core to
"owned keys first" (pi(r) = 2r+par for r<256, 2(r-256)+(1-par) otherwise).
Queries and keys share the axis, so one fp8 x tensor serves all three
projections: K/V projections read the leading 256 columns of each block
(the owned keys) directly via strided APs -- no separate gathered copy. The
host un-permutes the output columns.

On-device layout: scores are computed transposed, S^T[key, q]:
  - projections run in fp8e4 DoubleRow (2 contraction k-tiles per pass),
    outputs cast to bf16 (K^T, Q^T) / fp8 (V)
  - S^T chunk = bf16 matmul(lhsT=K^T[:, chunk], rhs=Q^T[:, qblock])
  - chunks are processed in PAIRS: both score matmuls land in one 2-bank
    PSUM tile; ONE ScalarE exp (scale=1/sqrt(d)) covers the pair,
    amortizing the fixed activation overhead; output E is fp8
  - the causal mask (0/-240) is BUILT ON DEVICE during the DMA-bound
    startup (pi row-vector broadcast via a K=1 matmul, two fused DVE
    tensor_scalar ops) and deposited into the masked pair's PSUM by an
    identity matmul before the scores accumulate -- masking never leaves
    the tensor queue, and exp flushes masked scores to exactly 0 in fp8
  - numerator: ONE fp8 DoubleRow matmul per pair (lhsT = V chunk pair)
  - denominator: ONE fp8 DoubleRow matmul per pair with a one-hot
    stationary [128, 2, 8] routing the sum into row qg of a single
    persistent PSUM bank [8, 512] holding all 8 blocks' denominators
  - query block 0 runs E/V in bf16 (early rows are near-copies of V and
    cannot tolerate fp8 quantization; elsewhere softmax averaging
    suppresses it)
  - a short warmup matmul chain runs while the first DMA pieces land so
    the PE p-state is ramped when real work arrives; projection work is
    split into ~17 filler units drip-fed one per attention pair to keep
    the tensor queue continuously busy; big blocks run early, a small
    block drains the kernel
"""

import math

import numpy as np

B, S, DIN, DOUT = 4, 4096, 512, 128
NCORES = 8
TQ = 512            # query block size
NQB = S // TQ       # 8 query blocks per batch
KC = 128            # key chunk size
NKLOC = S // KC // 2  # 16 key chunks owned per core
SK = NKLOC * KC     # 2048 owned keys
NDC = DIN // 128    # 4 contraction chunks
RSQRT_D = 1.0 / math.sqrt(float(DOUT))
NWARM = 20

_cache = {}


def _build_nc():
    import concourse.bacc as bacc
    import concourse.mybir as mybir
    import concourse.tile as tile

    f32 = mybir.dt.float32
    bf = mybir.dt.bfloat16
    f8 = mybir.dt.float8e4
    DR = mybir.MatmulPerfMode.DoubleRow
    EXP = mybir.ActivationFunctionType.Exp
    ALU = mybir.AluOpType

    nc = bacc.Bacc(None, target_bir_lowering=False, debug=False)

    # ---- DRAM parameters ----
    xq8_d = nc.declare_dram_parameter("xq8", [128, NDC, S], f8, isOutput=False)
    xv16_d = nc.declare_dram_parameter("xv16", [128, NDC, 2 * KC], bf, isOutput=False)
    w8_d = nc.declare_dram_parameter("w8", [128, 3, NDC, DOUT], f8, isOutput=False)
    i8_d = nc.declare_dram_parameter("i8", [128, 128], f8, isOutput=False)
    wv16_d = nc.declare_dram_parameter("wv16", [128, NDC, DOUT], bf, isOutput=False)
    pib_d = nc.declare_dram_parameter("pib", [1, TQ], bf, isOutput=False)
    koff_d = nc.declare_dram_parameter("koff", [128, 2], f32, isOutput=False)
    oh8_d = nc.declare_dram_parameter("oh8", [128, 2, 8 * NQB], f8, isOutput=False)
    oh16_d = nc.declare_dram_parameter("oh16", [128, 8], bf, isOutput=False)
    numT = nc.declare_dram_parameter("numT", [DOUT, S], f32, isOutput=True)
    den = nc.declare_dram_parameter("den", [NQB, TQ], f32, isOutput=True)

    with tile.TileContext(nc) as tc:
        with (
            tc.tile_pool(name="persist", bufs=1) as persist,
            tc.tile_pool(name="pp", bufs=1, space="PSUM") as pp,
            tc.tile_pool(name="ps2", bufs=2, space="PSUM") as ps2,
            tc.tile_pool(name="pso", bufs=2, space="PSUM") as pso,
            tc.tile_pool(name="psd", bufs=1, space="PSUM") as psd,
            tc.tile_pool(name="et", bufs=4) as et,
            tc.tile_pool(name="ot", bufs=2) as ot,
            tc.tile_pool(name="dt", bufs=1) as dt_pool,
        ):
            # ---- resident SBUF tensors ----
            xq8_t = persist.tile([128, NDC, S], f8, tag="xq8")
            xv16_t = persist.tile([128, NDC, 2 * KC], bf, tag="xv16")
            w8_t = persist.tile([128, 3, NDC, DOUT], f8, tag="w8")
            i8_t = persist.tile([128, 128], f8, tag="i8")
            wv16_t = persist.tile([128, NDC, DOUT], bf, tag="wv16")
            pib_t = persist.tile([1, TQ], bf, tag="pib")
            koff_t = persist.tile([128, 2], f32, tag="koff")
            mk8_t = persist.tile([128, 2, TQ], f8, tag="mk8")
            oh8_t = persist.tile([128, 2, 8 * NQB], f8, tag="oh8")
            oh16_t = persist.tile([128, 8], bf, tag="oh16")
            ones_r = persist.tile([1, 128], bf, tag="ones_r")
            mtmp = persist.tile([128, TQ], f32, tag="mtmp")
            qT = persist.tile([128, S], bf, tag="qT")
            kT = persist.tile([128, SK], bf, tag="kT")
            v8_t = persist.tile([128, NKLOC, DOUT], f8, tag="v8")
            v16_t = persist.tile([128, 2, DOUT], bf, tag="v16")

            # ---- input DMA: per-dma_start transfer rate is ~20-60 GB/s and
            # transfers on one issue queue serialize, so spread pieces over
            # all 3 queues, ordered by first-need (greedy by deadline).
            def xpiece(eng, a, b):
                eng.dma_start(out=xq8_t[:, :, a:b], in_=xq8_d[:, :, a:b])

            nc.sync.dma_start(out=w8_t[:, 1], in_=w8_d[:, 1])      # Wk
            nc.scalar.dma_start(out=pib_t[:], in_=pib_d[:])
            nc.scalar.dma_start(out=koff_t[:], in_=koff_d[:])
            nc.scalar.dma_start(out=w8_t[:, 0], in_=w8_d[:, 0])    # Wq
            nc.gpsimd.dma_start(out=w8_t[:, 2], in_=w8_d[:, 2])    # Wv
            xpiece(nc.sync, 0, 256)        # A1: blk0 owned (kproj0, qproj0)
            xpiece(nc.scalar, 512, 768)    # B: blk1 owned (kproj0)
            xpiece(nc.sync, 256, 512)      # A2: queries blk0
            nc.scalar.dma_start(out=oh8_t[:], in_=oh8_d[:])
            nc.scalar.dma_start(out=oh16_t[:], in_=oh16_d[:])
            nc.gpsimd.dma_start(out=xv16_t[:], in_=xv16_d[:])
            nc.gpsimd.dma_start(out=wv16_t[:], in_=wv16_d[:])
            nc.gpsimd.dma_start(out=i8_t[:], in_=i8_d[:])
            xpiece(nc.gpsimd, 768, 1024)   # B2: queries blk1
            xpiece(nc.sync, 1024, 1536)    # C: blk2 (kproj1, qproj2)
            xpiece(nc.scalar, 1536, 2048)  # D: blk3 (kproj1, qproj3)
            xpiece(nc.gpsimd, 2560, 3072)  # F: blk5 (kproj2, v10/11, qproj5)
            xpiece(nc.sync, 2048, 2560)    # E: blk4 (kproj2, v8/9, qproj4)
            xpiece(nc.scalar, 3584, S)     # H: blk7 (qproj7, kproj3)
            xpiece(nc.sync, 3072, 3584)    # G: blk6 (kproj3, qproj6)

            # ---- PE warmup: keep the tensor engine busy while the first
            # DMA pieces land so the p-state ramp is underway before real
            # work; operands are memset data, no DMA dependency (results
            # are never read)
            nc.vector.memset(ones_r[:], 1.0)
            warm = pso.tile([128, 128], f32, tag="po", name="warm")
            for w in range(NWARM):
                nc.tensor.matmul(
                    warm[:], ones_r[:], ones_r[:], start=True, stop=True,
                    skip_group_check=True,
                )

            # ---- on-device causal mask build (runs during DMA-bound start):
            # broadcast pi (biased by -256, bf16-exact) across partitions via
            # a K=1 matmul, then two fused DVE tensor_scalar ops per half:
            # mk8 = max(min(pi - koff, 0) * 240, -240) in {0, -240}
            pib_ps = psd.tile([128, TQ], f32, tag="pd", name="pib_ps")
            nc.tensor.matmul(pib_ps[:], ones_r[:], pib_t[:], start=True, stop=True)
            for c in (0, 1):
                nc.vector.tensor_scalar(
                    mtmp[:], pib_ps[:], koff_t[:, c:c + 1], 0.0,
                    op0=ALU.subtract, op1=ALU.min,
                )
                nc.vector.tensor_scalar(
                    mk8_t[:, c, :], mtmp[:], 240.0, -240.0,
                    op0=ALU.mult, op1=ALU.max,
                )

            # owned-key x columns: local chunk c lives at block floor(c/2),
            # column offset 128*(c%2)
            def xcol(c):
                return TQ * (c // 2) + KC * (c % 2)

            def kproj(g, ps=None):  # K^T for local chunks 4g..4g+3
                if ps is None:
                    ps = pp.tile([128, 512], f32, tag="pp", name=f"ppk{g}")
                xb = xq8_t.rearrange("p t (b c) -> p t b c", c=TQ)
                for j in (0, 1):
                    nc.tensor.matmul(
                        ps[:],
                        w8_t[:, 1, 2 * j:2 * j + 2, :],
                        xb[:, 2 * j:2 * j + 2, 2 * g:2 * g + 2, 0:2 * KC],
                        start=(j == 0),
                        stop=(j == 1),
                        perf_mode=DR,
                    )
                nc.vector.tensor_copy(kT[:, 512 * g:512 * (g + 1)], ps[:])

            def qproj(g, ps=None):  # Q^T for permuted queries [512g, 512g+512)
                if ps is None:
                    ps = pp.tile([128, 512], f32, tag="pp", name=f"ppq{g}")
                for j in (0, 1):
                    nc.tensor.matmul(
                        ps[:],
                        w8_t[:, 0, 2 * j:2 * j + 2, :],
                        xq8_t[:, 2 * j:2 * j + 2, 512 * g:512 * (g + 1)],
                        start=(j == 0),
                        stop=(j == 1),
                        perf_mode=DR,
                    )
                nc.vector.tensor_copy(qT[:, 512 * g:512 * (g + 1)], ps[:])

            def vproj2(g, ps=None):  # V for local chunks 2g, 2g+1, fp8
                if ps is None:
                    ps = pp.tile([128, 2, DOUT], f32, tag="pp", name=f"ppv{g}")
                for c in (0, 1):
                    ck = 2 * g + c
                    for j in (0, 1):
                        nc.tensor.matmul(
                            ps[:, c, :],
                            xq8_t[:, 2 * j:2 * j + 2, xcol(ck):xcol(ck) + KC],
                            w8_t[:, 2, 2 * j:2 * j + 2, :],
                            start=(j == 0),
                            stop=(j == 1),
                            perf_mode=DR,
                        )
                nc.vector.tensor_copy(v8_t[:, 2 * g:2 * (g + 1), :], ps[:])

            def v16proj():  # bf16 V for local chunks 0,1 (block-0 accuracy)
                ps = pp.tile([128, 2, DOUT], f32, tag="pp", name="ppv16")
                for c in (0, 1):
                    for t in range(NDC):
                        nc.tensor.matmul(
                            ps[:, c, :],
                            xv16_t[:, t, KC * c:KC * (c + 1)],
                            wv16_t[:, t, :],
                            start=(t == 0),
                            stop=(t == NDC - 1),
                        )
                nc.vector.tensor_copy(v16_t[:], ps[:])

            pd = psd.tile([8, TQ], f32, tag="pd", name="pd")

            def attn_block(qg, filler, out_engines, last_den_blk=False):
                npairs = qg + 1
                po = pso.tile([128, TQ], f32, tag="po", name=f"po{qg}")
                for i in range(npairs):
                    # one projection filler unit per pair keeps the tensor
                    # queue fed while exp runs (p-state stays ramped)
                    if filler:
                        filler.pop(0)()
                    masked = i == npairs - 1
                    pair = ps2.tile([128, 2, TQ], f32, tag="ps2", name=f"ps{qg}_{i}")
                    for c in (0, 1):
                        ck = 2 * i + c
                        if masked:
                            nc.tensor.matmul(
                                pair[:, c, :],
                                i8_t[:],
                                mk8_t[:, c, :],
                                start=True,
                                stop=False,
                                skip_group_check=True,
                            )
                        nc.tensor.matmul(
                            pair[:, c, :],
                            kT[:, KC * ck:KC * (ck + 1)],
                            qT[:, TQ * qg:TQ * (qg + 1)],
                            start=not masked,
                            stop=True,
                            skip_group_check=masked,
                        )
                    edt = bf if qg == 0 else f8
                    etag = "e16" if qg == 0 else "e8"
                    e = et.tile([128, 2, TQ], edt, tag=etag, name=f"e{qg}_{i}")
                    nc.scalar.activation(e[:], pair[:], EXP, scale=RSQRT_D)
                    last_den = last_den_blk and i == npairs - 1
                    if qg == 0:
                        for c in (0, 1):
                            nc.tensor.matmul(
                                po[:],
                                v16_t[:, c, :],
                                e[:, c, :],
                                start=(c == 0),
                                stop=(c == 1),
                            )
                            nc.tensor.matmul(
                                pd[:],
                                oh16_t[:],
                                e[:, c, :],
                                start=(c == 0),
                                stop=False,
                                skip_group_check=True,
                            )
                    else:
                        nc.tensor.matmul(
                            po[:],
                            v8_t[:, 2 * i:2 * i + 2, :],
                            e[:],
                            start=(i == 0),
                            stop=(i == npairs - 1),
                            perf_mode=DR,
                        )
                        nc.tensor.matmul(
                            pd[:],
                            oh8_t[:, :, 8 * qg:8 * (qg + 1)],
                            e[:],
                            start=False,
                            stop=last_den,
                            perf_mode=DR,
                            skip_group_check=True,
                        )
                o = ot.tile([128, TQ], f32, tag="o", name=f"o{qg}")
                nc.vector.tensor_copy(o[:], po[:])
                # split the output DMA across the given (idle) issue queues
                # so the drain of the final blocks doesn't serialize on one
                nsp = len(out_engines)
                w = TQ // nsp
                for s_i, eng in enumerate(out_engines):
                    eng.dma_start(
                        out=numT[:, TQ * qg + s_i * w:TQ * qg + (s_i + 1) * w],
                        in_=o[:, s_i * w:(s_i + 1) * w],
                    )

            # ---- schedule: upfront projections for block 0 run in the (still
            # idle) score-pair PSUM slots so they pipeline without waiting on
            # the single pp bank; then big blocks early / small block last,
            # with the remaining projection work drip-fed one unit per
            # attention pair (deadline-ordered)
            kq = ps2.tile([128, 2, TQ], f32, tag="ps2", name="up_kq")
            vv = ps2.tile([128, 2, TQ], f32, tag="ps2", name="up_vv")
            kproj(0, ps=kq[:, 0, :])
            qproj(0, ps=kq[:, 1, :])
            vproj2(0, ps=vv[:, :, 0:DOUT])
            vproj2(1, ps=vv[:, :, DOUT:2 * DOUT])
            filler = [
                v16proj,
                lambda: qproj(1),
                lambda: qproj(2),
                lambda: kproj(1),
                lambda: vproj2(2),
                lambda: qproj(3),
                lambda: vproj2(3),
                lambda: qproj(7),
                lambda: kproj(2),
                lambda: vproj2(4),
                lambda: vproj2(5),
                lambda: kproj(3),
                lambda: vproj2(6),
                lambda: vproj2(7),
                lambda: qproj(6),
                lambda: qproj(5),
                lambda: qproj(4),
            ]
            gp, sy, sc = nc.gpsimd, nc.sync, nc.scalar
            attn_block(0, filler, [gp])
            attn_block(1, filler, [gp])
            attn_block(2, filler, [gp])
            attn_block(3, filler, [gp])
            attn_block(7, filler, [gp])
            attn_block(6, filler, [gp])
            attn_block(5, filler, [sy])
            attn_block(4, filler, [sy, sc], last_den_blk=True)

            d = dt_pool.tile([8, TQ], f32, tag="d", name="d")
            nc.vector.tensor_copy(d[:], pd[:])
            nc.sync.dma_start(out=den[:, :], in_=d[:])

    nc.finalize()
    return nc


def _perm(par):
    # within-block permutation: owned keys first.
    # pi[r] = original offset of permuted position r
    r = np.arange(TQ)
    return np.where(r < TQ // 2, 2 * r + par, 2 * (r - TQ // 2) + (1 - par))


def _get_nc():
    if "nc" not in _cache:
        _cache["nc"] = _build_nc()
    return _cache["nc"]


def _pack_pm(a):
    # [DIN, cols] -> partition-major [128, DIN//128, cols]
    return np.ascontiguousarray(a.reshape(DIN // 128, 128, a.shape[1]).transpose(1, 0, 2))


def _prepare_in_maps(x, Wq, Wk, Wv):
    import ml_dtypes

    f8 = ml_dtypes.float8_e4m3
    bf = ml_dtypes.bfloat16

    # [128, 3, NDC, DOUT]: w8[p, i, c, e] = W_i[128c + p, e]
    w8 = np.stack([_pack_pm(w).reshape(128, NDC, DOUT) for w in (Wq, Wk, Wv)], axis=1)
    w8 = np.ascontiguousarray(w8).astype(f8)
    wv16 = _pack_pm(Wv).astype(bf)
    i8 = np.eye(128, dtype=np.float32).astype(f8)

    # one-hot denominator routers
    oh8 = np.zeros((128, 2, 8 * NQB), dtype=np.float32)
    for qg in range(NQB):
        oh8[:, :, 8 * qg + qg] = 1.0
    oh8 = oh8.astype(f8)
    oh16 = np.zeros((128, 8), dtype=np.float32)
    oh16[:, 0] = 1.0
    oh16 = oh16.astype(bf)

    in_maps = []
    for c in range(NCORES):
        b, par = c // 2, c % 2
        pi = _perm(par)
        base = (np.arange(S) // TQ) * TQ
        xp = x[b].T.astype(np.float32)[:, base + pi[np.arange(S) % TQ]]
        # mask build operands: pib = pi - 256 (bf16-exact); koff[p, c] =
        # 256c + 2p + par - 256
        pib = (pi.astype(np.float32) - 256.0)[None, :].astype(bf)
        p_idx = np.arange(128, dtype=np.float32)
        koff = np.stack(
            [256.0 * cc + 2.0 * p_idx + par - 256.0 for cc in (0, 1)], axis=1
        ).astype(np.float32)
        in_maps.append({
            "xq8": _pack_pm(xp).astype(f8),
            "xv16": _pack_pm(np.ascontiguousarray(xp[:, :2 * KC])).astype(bf),
            "w8": w8,
            "i8": i8,
            "wv16": wv16,
            "pib": pib,
            "koff": koff,
            "oh8": oh8,
            "oh16": oh16,
        })
    return in_maps


def _gather(results):
    out = np.empty((B, S, DOUT), dtype=np.float32)
    base = (np.arange(S) // TQ) * TQ
    for b in range(B):
        acc_num = np.zeros((S, DOUT), dtype=np.float64)
        acc_den = np.zeros(S, dtype=np.float64)
        for par in range(2):
            r = results[2 * b + par]
            gidx = base + _perm(par)[np.arange(S) % TQ]  # permuted pos -> orig
            acc_num[gidx] += r["numT"].astype(np.float64).T
            acc_den[gidx] += r["den"].astype(np.float64).reshape(-1)
        out[b] = (acc_num / acc_den[:, None]).astype(np.float32)
    return out


def kernel(**inputs):
    from concourse.bass_utils import run_bass_kernel_spmd

    x = np.asarray(inputs["x"], dtype=np.float32)
    Wq = np.asarray(inputs["Wq"], dtype=np.float32)
    Wk = np.asarray(inputs["Wk"], dtype=np.float32)
    Wv = np.asarray(inputs["Wv"], dtype=np.float32)

    nc = _get_nc()
    in_maps = _prepare_in_maps(x, Wq, Wk, Wv)
    res = run_bass_kernel_spmd(nc, in_maps, list(range(NCORES)))
    return _gather(res.results)
